# revision 12
# baseline (speedup 1.0000x reference)
"""Multi-head graph-attention layer for Trainium2 (8-core SPMD).

The reference computes per-head projections hp = einsum("bnf,hfd->bhnd", h, W),
dense attention scores e = hp @ hp^T, LeakyReLU, softmax over the last axis,
and then multiplies hp by sum_j(softmax(e))_j. The sum of a softmax over its
own normalization axis is identically 1, so the layer's exact mathematical
output is hp itself (concatenated over heads):

    out[b, n, h*64+d] = sum_f h[b,n,f] * W[h,f,d]  =  (h[b] @ Wc)[n, h*64+d]

with Wc[f, h*64+d] = W[h,f,d]. `adj` is unused by the reference and ignored.

Sharding: data-parallel over the batch dim B=8, one graph per NeuronCore.
Each core computes Y[b]^T = (Wc^T @ h[b]^T) as a [256,256] x [256,2048]
matmul in bf16 (measured rel err ~2.8e-3 vs the 2e-2 gate), halving both
input and output DMA bytes vs fp32: the kernel is DMA-bound.

HW model distilled from traces (per core):
- DMA engines process ~one descriptor-packet per ~80ns each; packets cap
  at 2048 B, so rows that are multiples of 2048 B move at the full
  ~400 GB/s aggregate while shorter rows waste packet slots. Every DMA
  also costs ~0.6us sequencer issue + ~0.7us DGE start latency + ~0.9us
  completion-semaphore propagation, and >=128 descriptors (~0.64us).
- PE reaches 2.4 GHz only after ~3us of continuous busy (cold 2-4x
  slower), hence scratch warm-up/filler matmuls.

Pipeline (per core):
- Host packs one [128, 4608] bf16 input: Wc's two 128-row k-chunks in
  cols [0:512), then per node-range r its Xk0|Xk1 blocks side by side,
  so one chunk DMA delivers everything a matmul group needs. Chunks are
  1024 cols (2048 B rows) in need-order on the sync HWDGE queue (FIFO =
  input keeps bus priority over the outputs queued behind it).
- Node ranges 256/512/512/512/256: the small head range starts the PE
  ~0.4us earlier; the small tail range shortens the last
  mm->evict->issue->transfer chain. Ranges 0 and 4 share a PSUM bank
  per m so 2m x 4 banks exactly fits PSUM.
- PSUM eviction converts fp32->bf16 on the fly: DVE evicts m0, ACT
  evicts m1 in parallel into a shared [128,4096] bf16 y tile; sync
  issues the first four output DMAs, ACT issues the last one itself
  right after its final evict (skips queueing behind sync).
"""

import numpy as np

import concourse.bass as bass
import concourse.mybir as mybir
import concourse.tile as tile
from concourse import bacc
from concourse.bass_utils import run_bass_kernel_spmd

B = 8          # graphs == cores
N = 2048       # nodes per graph
F_IN = 256     # input features (= contraction dim K)
F_OUT = 256    # num_heads * d_head
P = 128        # SBUF/PSUM partitions
KC = 2         # contraction chunks (256 = 2*128)
MC = 2         # output-feature chunks (256 = 2*128)

WCOLS = 512            # cols [0:512) hold Wc k0|k1
XIN_COLS = WCOLS + 2 * N   # 4608
YT_COLS = 2 * N            # 4096

# (node_c0, width, psum_bank, psum_lo): compute order. Ranges 0 and 4
# share psum bank 0 ([0:256) and [256:512)).
MM_RANGES = [
    (0, 256, 0, 0),
    (256, 512, 1, 0),
    (768, 512, 2, 0),
    (1280, 512, 3, 0),
    (1792, 256, 0, 256),
]
# input DMA chunk column bounds (xin cols); chunk i feeds range i
CHUNKS = [(0, 1024), (1024, 2048), (2048, 3072), (3072, 4096), (4096, 4608)]

# scratch matmul free-dims covering the first input-DMA wait; the HW clock
# manager boosts the PE only after ~4.5us of sustained activity, so start
# as early as possible and stay busy until the first chunk lands.
WARMUPS = [512, 512, 512, 512, 256]
FILLERS = [0, 1, 1, 1, 1]  # 256-free PE keep-warm matmuls before each range

_module_cache = {}

# test.py reads this after calling kernel() to get profile/exec-time info.
LAST_RESULTS = None


def _xcols(r):
    """(k0_col, k1_col) start columns of range r's X blocks in xin."""
    c0, w, _, _ = MM_RANGES[r]
    s = WCOLS + 2 * c0
    return s, s + w


def _ylo(r):
    """start col of range r's [m0|m1] block in yt."""
    return 2 * MM_RANGES[r][0]


def _build_module() -> bass.Bass:
    in_dt = mybir.dt.bfloat16

    nc = bacc.Bacc(None, target_bir_lowering=False, enable_partition_id=False)
    xin = nc.dram_tensor("xin", [P, XIN_COLS], in_dt, kind="ExternalInput")
    yt = nc.dram_tensor("yt", [P, YT_COLS], mybir.dt.bfloat16, kind="ExternalOutput")

    with tile.TileContext(nc) as tc:
        with (
            tc.tile_pool(name="sbpool", bufs=1) as sbpool,
            tc.tile_pool(name="pspool", bufs=1, space="PSUM") as pspool,
        ):
            # Scratch operands for PE warm-up (values irrelevant, but Tile
            # requires a write). DVE memset is fast and DVE is otherwise
            # idle here, so the warm-up matmuls start ~0.5us earlier than
            # with the gpsimd memset, pulling the HW clock boost forward.
            wu = sbpool.tile([P, 512], mybir.dt.bfloat16, name="wu", tag="wu")
            nc.vector.memset(wu[:], 0.0)
            wu_mm = wu[:]

            # ctx-index tiles for the SWDGE kv_writeback path of the last
            # two output blocks (see below): value = block start col in yt.
            idx_wb = []
            for r in (3, 4):
                t = sbpool.tile([P, 1], mybir.dt.int32, name=f"idx{r}", tag=f"idx{r}")
                nc.gpsimd.memset(t[:], _ylo(r))
                idx_wb.append(t)
            wb_sems = [nc.alloc_semaphore(f"wb{r}") for r in (3, 4)]

            x_sb = sbpool.tile([P, XIN_COLS], in_dt, name="x", tag="x")
            y_sb = sbpool.tile([P, YT_COLS], mybir.dt.bfloat16, name="y", tag="y")

            # Input chunk DMAs in need-order on the sync HWDGE queue.
            for lo, hi in CHUNKS:
                nc.sync.dma_start(x_sb[:, lo:hi], xin[:, lo:hi])

            ps = [
                [
                    pspool.tile([P, 512], mybir.dt.float32, name=f"ps{m}_{j}", tag=f"ps{m}_{j}")
                    for j in range(4)
                ]
                for m in range(MC)
            ]

            def ps_slice(m, r):
                _, w, j, lo = MM_RANGES[r]
                return ps[m][j][:, lo : lo + w]

            # PE clock warm-up on scratch data while the first chunk is in
            # flight (targets ps[1][3]: real accumulation there starts last
            # among full banks; Tile's WAW tracking keeps program order).
            for wfree in WARMUPS:
                nc.tensor.matmul(
                    ps[1][3][:, :wfree], wu_mm[:, :P], wu_mm[:, :wfree],
                    start=True, stop=True,
                )

            def filler(r, count):
                # Keep the PE busy during input-wait gaps (256-free, cheap).
                # Target a bank that is not mid-accumulation at that point
                # in program order.
                if r <= 2:
                    tgt = ps[1][3][:, 0:256]               # bank3 still unused
                elif r == 3:
                    tgt = ps[0][0][:, 0:256]               # r0 m0 evicted
                else:
                    tgt = ps[0][1][:, 0:256]               # r1 m0 evicted
                for _ in range(count):
                    nc.tensor.matmul(tgt, wu_mm[:, :P], wu_mm[:, :256], start=True, stop=True)

            for r, (c0, w, _, _) in enumerate(MM_RANGES):
                k0c, k1c = _xcols(r)
                filler(r, FILLERS[r])
                for k, kc in enumerate((k0c, k1c)):
                    for m in range(MC):
                        nc.tensor.matmul(
                            ps_slice(m, r),
                            x_sb[:, k * F_OUT + m * P : k * F_OUT + (m + 1) * P],
                            x_sb[:, kc : kc + w],
                            start=(k == 0),
                            stop=(k == KC - 1),
                        )
                # Evict fp32 PSUM -> bf16 SBUF: DVE does m0, ACT does m1 in
                # parallel; one output DMA per range for the packed [m0|m1]
                # block. The first three blocks go out as HWDGE DMAs on
                # sync's queue (FIFO behind the input = input keeps bus
                # priority). The last two blocks instead use SWDGE
                # kv_writeback descriptors PREPARED EARLY on the idle Pool
                # engine and FIRED by one cheap trigger_dma after the final
                # evicts - skipping the ~0.6us HWDGE issue + ~0.7us DGE
                # start latency that would otherwise sit on the tail's
                # critical chain. (kv_writeback with batch=1, d_head=128,
                # dho=1, n_ctx=4096 writes yt[p, ctx:ctx+ncn] = src[p, :],
                # with ctx from the per-block idx tile.)
                ylo = _ylo(r)
                nc.vector.tensor_copy(y_sb[:, ylo : ylo + w], ps_slice(0, r))
                nc.scalar.copy(y_sb[:, ylo + w : ylo + 2 * w], ps_slice(1, r))
                if r >= 3:
                    nc.gpsimd.kv_writeback(
                        yt[:, :].rearrange("p (o c) -> () p o c", o=1),
                        y_sb[:, ylo : ylo + 2 * w].rearrange(
                            "p (o b c) -> p o b c", o=1, b=1
                        ),
                        idx_wb[r - 3][:],
                        prepare_only=True,
                        sem=wb_sems[r - 3],
                    )
                    if r == len(MM_RANGES) - 1:
                        nc.gpsimd.trigger_dma(count=None)
                else:
                    nc.sync.dma_start(
                        yt[:, ylo : ylo + 2 * w], y_sb[:, ylo : ylo + 2 * w]
                    )
    nc.compile()
    return nc


def _get_module() -> bass.Bass:
    if "m" not in _module_cache:
        _module_cache["m"] = _build_module()
    return _module_cache["m"]


def kernel(h: np.ndarray, adj: np.ndarray, W: np.ndarray, **_unused) -> np.ndarray:
    global LAST_RESULTS
    import ml_dtypes

    bf16 = ml_dtypes.bfloat16
    h = np.asarray(h, dtype=np.float32)
    W = np.asarray(W, dtype=np.float32)
    # Wc[f, head*64+d] = W[head, f, d]
    wc = np.ascontiguousarray(W.transpose(1, 0, 2).reshape(F_IN, F_OUT)).astype(bf16)

    in_maps = []
    for b in range(B):
        xt = h[b].T.astype(bf16)  # [256 f, 2048 n]
        xin = np.empty((P, XIN_COLS), dtype=bf16)
        xin[:, 0:F_OUT] = wc[0:P]
        xin[:, F_OUT : 2 * F_OUT] = wc[P : 2 * P]
        for r, (c0, w, _, _) in enumerate(MM_RANGES):
            s, s1 = _xcols(r)
            xin[:, s : s + w] = xt[0:P, c0 : c0 + w]
            xin[:, s1 : s1 + w] = xt[P : 2 * P, c0 : c0 + w]
        in_maps.append({"xin": xin})

    nc = _get_module()
    res = run_bass_kernel_spmd(nc, in_maps, core_ids=list(range(B)))
    LAST_RESULTS = res

    out = np.empty((B, N, F_OUT), dtype=np.float32)
    yt_full = np.empty((F_OUT, N), dtype=np.float32)
    for b in range(B):
        ytb = res.results[b]["yt"]
        for r, (c0, w, _, _) in enumerate(MM_RANGES):
            ylo = _ylo(r)
            blk = ytb[:, ylo : ylo + 2 * w].astype(np.float32)
            yt_full[0:P, c0 : c0 + w] = blk[:, 0:w]
            yt_full[P : 2 * P, c0 : c0 + w] = blk[:, w : 2 * w]
        out[b] = yt_full.T
    return out


# revision 16
# speedup vs baseline: 1.0344x; 1.0344x over previous
"""Multi-head graph-attention layer for Trainium2 (8-core SPMD).

The reference computes per-head projections hp = einsum("bnf,hfd->bhnd", h, W),
dense attention scores e = hp @ hp^T, LeakyReLU, softmax over the last axis,
and then multiplies hp by sum_j(softmax(e))_j. The sum of a softmax over its
own normalization axis is identically 1, so the layer's exact mathematical
output is hp itself (concatenated over heads):

    out[b, n, h*64+d] = sum_f h[b,n,f] * W[h,f,d]  =  (h[b] @ Wc)[n, h*64+d]

with Wc[f, h*64+d] = W[h,f,d]. `adj` is unused by the reference and ignored.

Sharding: data-parallel over the batch dim B=8, one graph per NeuronCore.
Each core computes Y[b]^T = (Wc^T @ h[b]^T) as a [256,256] x [256,2048]
matmul in bf16 (measured rel err ~2.8e-3 vs the 2e-2 gate), halving both
input and output DMA bytes vs fp32: the kernel is DMA-bound.

HW model distilled from traces (per core):
- DMA engines process ~one descriptor-packet per ~80ns each; packets cap
  at 2048 B, so rows that are multiples of 2048 B move at the full
  ~400 GB/s aggregate while shorter rows waste packet slots. Every DMA
  also costs ~0.6us sequencer issue + ~0.7us DGE start latency + ~0.9us
  completion-semaphore propagation, and >=128 descriptors (~0.64us).
- PE reaches 2.4 GHz only after ~3us of continuous busy (cold 2-4x
  slower), hence scratch warm-up/filler matmuls.

Pipeline (per core):
- Host packs one [128, 4608] bf16 input: Wc's two 128-row k-chunks in
  cols [0:512), then per node-range r its Xk0|Xk1 blocks side by side,
  so one chunk DMA delivers everything a matmul group needs. Chunks are
  1024 cols (2048 B rows) in need-order on the sync HWDGE queue (FIFO =
  input keeps bus priority over the outputs queued behind it).
- Node ranges 256/512/512/512/256: the small head range starts the PE
  ~0.4us earlier; the small tail range shortens the last
  mm->evict->issue->transfer chain. Ranges 0 and 4 share a PSUM bank
  per m so 2m x 4 banks exactly fits PSUM.
- PSUM eviction converts fp32->bf16 on the fly: DVE evicts m0, ACT
  evicts m1 in parallel into a shared [128,4096] bf16 y tile; sync
  issues the first four output DMAs, ACT issues the last one itself
  right after its final evict (skips queueing behind sync).
"""

import numpy as np

import concourse.bass as bass
import concourse.mybir as mybir
import concourse.tile as tile
from concourse import bacc
from concourse.bass_utils import run_bass_kernel_spmd

B = 8          # graphs == cores
N = 2048       # nodes per graph
F_IN = 256     # input features (= contraction dim K)
F_OUT = 256    # num_heads * d_head
P = 128        # SBUF/PSUM partitions
KC = 2         # contraction chunks (256 = 2*128)
MC = 2         # output-feature chunks (256 = 2*128)

WCOLS = 512            # cols [0:512) hold Wc k0|k1
XIN_COLS = WCOLS + 2 * N   # 4608
YT_COLS = 2 * N            # 4096

# (node_c0, width, psum_bank, psum_lo): compute order. Ranges 0 and 4
# share psum bank 0 ([0:256) and [256:512)).
MM_RANGES = [
    (0, 256, 0, 0),
    (256, 512, 1, 0),
    (768, 512, 2, 0),
    (1280, 512, 3, 0),
    (1792, 256, 0, 256),
]
# input DMA chunk column bounds (xin cols); chunk i feeds range i
CHUNKS = [(0, 1024), (1024, 2048), (2048, 3072), (3072, 4096), (4096, 4608)]

# scratch matmul free-dims covering the first input-DMA wait; the HW clock
# manager boosts the PE only after ~4.5us of sustained activity, so start
# as early as possible and stay busy until the first chunk lands.
WARMUPS = [512, 512, 512, 512, 256]
FILLERS = [0, 1, 1, 1, 1]  # 256-free PE keep-warm matmuls before each range

_module_cache = {}

# test.py reads this after calling kernel() to get profile/exec-time info.
LAST_RESULTS = None


def _xcols(r):
    """(k0_col, k1_col) start columns of range r's X blocks in xin."""
    c0, w, _, _ = MM_RANGES[r]
    s = WCOLS + 2 * c0
    return s, s + w


def _ylo(r):
    """start col of range r's [m0|m1] block in yt."""
    return 2 * MM_RANGES[r][0]


def _build_module() -> bass.Bass:
    in_dt = mybir.dt.bfloat16

    nc = bacc.Bacc(None, target_bir_lowering=False, enable_partition_id=False)
    xin = nc.dram_tensor("xin", [P, XIN_COLS], in_dt, kind="ExternalInput")
    # Blocks r0-r2 share one output tensor (HWDGE DMAs); r3a/r3b get their
    # own tensors so the two kv_writeback preps have disjoint dst claims
    # (a shared dst would WAW-chain prep_b behind prep_a's DMA completion,
    # which only happens at trigger time - a cycle that serializes
    # everything). ctx idx is 0 into each private tensor.
    yt = nc.dram_tensor("yt", [P, 2 * 1536], mybir.dt.bfloat16, kind="ExternalOutput")
    yta = nc.dram_tensor("yta", [P, 2 * 512], mybir.dt.bfloat16, kind="ExternalOutput")
    ytb = nc.dram_tensor("ytb", [P, 2 * 256], mybir.dt.bfloat16, kind="ExternalOutput")
    wb_dst = {3: yta, 4: ytb}

    with tile.TileContext(nc) as tc:
        with (
            tc.tile_pool(name="sbpool", bufs=1) as sbpool,
            tc.tile_pool(name="pspool", bufs=1, space="PSUM") as pspool,
        ):
            # Scratch operands for PE warm-up (values irrelevant, but Tile
            # requires a write). DVE memset is fast and DVE is otherwise
            # idle here, so the warm-up matmuls start ~0.5us earlier than
            # with the gpsimd memset, pulling the HW clock boost forward.
            wu = sbpool.tile([P, 512], mybir.dt.bfloat16, name="wu", tag="wu")
            nc.vector.memset(wu[:], 0.0)
            wu_mm = wu[:]

            # ctx-index tile (value 0) for the SWDGE kv_writeback path of
            # the last two output blocks (each writes its own dst tensor
            # from position 0).
            idx_wb = sbpool.tile([P, 1], mybir.dt.int32, name="idx0", tag="idx0")
            nc.gpsimd.memset(idx_wb[:], 0)
            wb_sems = [nc.alloc_semaphore(f"wb{r}") for r in (3, 4)]

            x_sb = sbpool.tile([P, XIN_COLS], in_dt, name="x", tag="x")
            y_sb = sbpool.tile([P, YT_COLS], mybir.dt.bfloat16, name="y", tag="y")

            # Input chunk DMAs in need-order on the sync HWDGE queue.
            for lo, hi in CHUNKS:
                nc.sync.dma_start(x_sb[:, lo:hi], xin[:, lo:hi])

            ps = [
                [
                    pspool.tile([P, 512], mybir.dt.float32, name=f"ps{m}_{j}", tag=f"ps{m}_{j}")
                    for j in range(4)
                ]
                for m in range(MC)
            ]

            def ps_slice(m, r):
                _, w, j, lo = MM_RANGES[r]
                return ps[m][j][:, lo : lo + w]

            # PE clock warm-up on scratch data while the first chunk is in
            # flight (targets ps[1][3]: real accumulation there starts last
            # among full banks; Tile's WAW tracking keeps program order).
            for wfree in WARMUPS:
                nc.tensor.matmul(
                    ps[1][3][:, :wfree], wu_mm[:, :P], wu_mm[:, :wfree],
                    start=True, stop=True,
                )

            def filler(r, count):
                # Keep the PE busy during input-wait gaps (256-free, cheap).
                # Target a bank that is not mid-accumulation at that point
                # in program order.
                if r <= 2:
                    tgt = ps[1][3][:, 0:256]               # bank3 still unused
                elif r == 3:
                    tgt = ps[0][0][:, 0:256]               # r0 m0 evicted
                else:
                    tgt = ps[0][1][:, 0:256]               # r1 m0 evicted
                for _ in range(count):
                    nc.tensor.matmul(tgt, wu_mm[:, :P], wu_mm[:, :256], start=True, stop=True)

            for r, (c0, w, _, _) in enumerate(MM_RANGES):
                k0c, k1c = _xcols(r)
                filler(r, FILLERS[r])
                for k, kc in enumerate((k0c, k1c)):
                    for m in range(MC):
                        nc.tensor.matmul(
                            ps_slice(m, r),
                            x_sb[:, k * F_OUT + m * P : k * F_OUT + (m + 1) * P],
                            x_sb[:, kc : kc + w],
                            start=(k == 0),
                            stop=(k == KC - 1),
                        )
                # Evict fp32 PSUM -> bf16 SBUF: DVE does m0, ACT does m1 in
                # parallel; one output DMA per range for the packed [m0|m1]
                # block. The first three blocks go out as HWDGE DMAs on
                # sync's queue (FIFO behind the input = input keeps bus
                # priority). The last two blocks instead use SWDGE
                # kv_writeback descriptors PREPARED EARLY on the idle Pool
                # engine and FIRED by one cheap trigger_dma after the final
                # evicts - skipping the ~0.6us HWDGE issue + ~0.7us DGE
                # start latency that would otherwise sit on the tail's
                # critical chain. (kv_writeback with batch=1, d_head=128,
                # dho=1, n_ctx=4096 writes yt[p, ctx:ctx+ncn] = src[p, :],
                # with ctx from the per-block idx tile.)
                ylo = _ylo(r)
                nc.vector.tensor_copy(y_sb[:, ylo : ylo + w], ps_slice(0, r))
                nc.scalar.copy(y_sb[:, ylo + w : ylo + 2 * w], ps_slice(1, r))
                if r >= 3:
                    nc.gpsimd.kv_writeback(
                        wb_dst[r][:, :].rearrange("p (o c) -> () p o c", o=1),
                        y_sb[:, ylo : ylo + 2 * w].rearrange(
                            "p (o b c) -> p o b c", o=1, b=1
                        ),
                        idx_wb[:],
                        prepare_only=True,
                        sem=wb_sems[r - 3],
                    )
                    if r == len(MM_RANGES) - 1:
                        nc.gpsimd.trigger_dma(count=None)
                else:
                    nc.sync.dma_start(
                        yt[:, ylo : ylo + 2 * w], y_sb[:, ylo : ylo + 2 * w]
                    )
    nc.compile()
    return nc


def _get_module() -> bass.Bass:
    if "m" not in _module_cache:
        _module_cache["m"] = _build_module()
    return _module_cache["m"]


def kernel(h: np.ndarray, adj: np.ndarray, W: np.ndarray, **_unused) -> np.ndarray:
    global LAST_RESULTS
    import ml_dtypes

    bf16 = ml_dtypes.bfloat16
    h = np.asarray(h, dtype=np.float32)
    W = np.asarray(W, dtype=np.float32)
    # Wc[f, head*64+d] = W[head, f, d]
    wc = np.ascontiguousarray(W.transpose(1, 0, 2).reshape(F_IN, F_OUT)).astype(bf16)

    in_maps = []
    for b in range(B):
        xt = h[b].T.astype(bf16)  # [256 f, 2048 n]
        xin = np.empty((P, XIN_COLS), dtype=bf16)
        xin[:, 0:F_OUT] = wc[0:P]
        xin[:, F_OUT : 2 * F_OUT] = wc[P : 2 * P]
        for r, (c0, w, _, _) in enumerate(MM_RANGES):
            s, s1 = _xcols(r)
            xin[:, s : s + w] = xt[0:P, c0 : c0 + w]
            xin[:, s1 : s1 + w] = xt[P : 2 * P, c0 : c0 + w]
        in_maps.append({"xin": xin})

    nc = _get_module()
    res = run_bass_kernel_spmd(nc, in_maps, core_ids=list(range(B)))
    LAST_RESULTS = res

    out = np.empty((B, N, F_OUT), dtype=np.float32)
    yt_full = np.empty((F_OUT, N), dtype=np.float32)
    for b in range(B):
        r_res = res.results[b]
        for r, (c0, w, _, _) in enumerate(MM_RANGES):
            if r == 3:
                blk = r_res["yta"].astype(np.float32)
            elif r == 4:
                blk = r_res["ytb"].astype(np.float32)
            else:
                ylo = _ylo(r)
                blk = r_res["yt"][:, ylo : ylo + 2 * w].astype(np.float32)
            yt_full[0:P, c0 : c0 + w] = blk[:, 0:w]
            yt_full[P : 2 * P, c0 : c0 + w] = blk[:, w : 2 * w]
        out[b] = yt_full.T
    return out


# revision 18
# speedup vs baseline: 1.2519x; 1.2103x over previous
"""Multi-head graph-attention layer for Trainium2 (8-core SPMD).

The reference computes per-head projections hp = einsum("bnf,hfd->bhnd", h, W),
dense attention scores e = hp @ hp^T, LeakyReLU, softmax over the last axis,
and then multiplies hp by sum_j(softmax(e))_j. The sum of a softmax over its
own normalization axis is identically 1, so the layer's exact mathematical
output is hp itself (concatenated over heads):

    out[b, n, h*64+d] = sum_f h[b,n,f] * W[h,f,d]  =  (h[b] @ Wc)[n, h*64+d]

with Wc[f, h*64+d] = W[h,f,d]. `adj` is unused by the reference and ignored.

Sharding: data-parallel over the batch dim B=8, one graph per NeuronCore.
Each core computes Y[b]^T = (Wc^T @ h[b]^T) as a [256,256] x [256,2048]
matmul in bf16 (measured rel err ~2.8e-3 vs the 2e-2 gate), halving both
input and output DMA bytes vs fp32: the kernel is DMA-bound.

HW model distilled from traces (per core):
- DMA engines process ~one descriptor-packet per ~80ns each; packets cap
  at 2048 B, so rows that are multiples of 2048 B move at the full
  ~400 GB/s aggregate while shorter rows waste packet slots. Every DMA
  also costs ~0.6us sequencer issue + ~0.7us DGE start latency + ~0.9us
  completion-semaphore propagation, and >=128 descriptors (~0.64us).
- PE reaches 2.4 GHz only after ~3us of continuous busy (cold 2-4x
  slower), hence scratch warm-up/filler matmuls.

Pipeline (per core):
- Host packs one [128, 4608] bf16 input: Wc's two 128-row k-chunks in
  cols [0:512), then per node-range r its Xk0|Xk1 blocks side by side,
  so one chunk DMA delivers everything a matmul group needs. Chunks are
  1024 cols (2048 B rows) in need-order on the sync HWDGE queue (FIFO =
  input keeps bus priority over the outputs queued behind it).
- Node ranges 256/512/512/512/256: the small head range starts the PE
  ~0.4us earlier; the small tail range shortens the last
  mm->evict->issue->transfer chain. Ranges 0 and 4 share a PSUM bank
  per m so 2m x 4 banks exactly fits PSUM.
- PSUM eviction converts fp32->bf16 on the fly: DVE evicts m0, ACT
  evicts m1 in parallel into a shared [128,4096] bf16 y tile; sync
  issues the first four output DMAs, ACT issues the last one itself
  right after its final evict (skips queueing behind sync).
"""

import numpy as np

import concourse.bass as bass
import concourse.mybir as mybir
import concourse.tile as tile
from concourse import bacc
from concourse.bass_utils import run_bass_kernel_spmd

B = 8          # graphs == cores
N = 2048       # nodes per graph
F_IN = 256     # input features (= contraction dim K)
F_OUT = 256    # num_heads * d_head
P = 128        # SBUF/PSUM partitions
KC = 2         # contraction chunks (256 = 2*128)
MC = 2         # output-feature chunks (256 = 2*128)

WCOLS = 512            # cols [0:512) hold Wc k0|k1
XIN_COLS = WCOLS + 2 * N   # 4608
YT_COLS = 2 * N            # 4096

# (node_c0, width, psum_bank, psum_lo): compute order. Ranges 0 and 4
# share psum bank 0 ([0:256) and [256:512)).
MM_RANGES = [
    (0, 256, 0, 0),
    (256, 512, 1, 0),
    (768, 512, 2, 0),
    (1280, 512, 3, 0),
    (1792, 256, 0, 256),
]
# input DMA chunk column bounds (xin cols); chunk i feeds range i
CHUNKS = [(0, 1024), (1024, 2048), (2048, 3072), (3072, 4096), (4096, 4608)]

# scratch matmul free-dims covering the first input-DMA wait; the HW clock
# manager boosts the PE only after ~4.5us of sustained activity, so start
# as early as possible and stay busy until the first chunk lands.
WARMUPS = [512, 512, 512, 512, 256]
FILLERS = [0, 1, 1, 1, 1]  # 256-free PE keep-warm matmuls before each range

_module_cache = {}

# test.py reads this after calling kernel() to get profile/exec-time info.
LAST_RESULTS = None


def _xcols(r):
    """(k0_col, k1_col) start columns of range r's X blocks in xin."""
    c0, w, _, _ = MM_RANGES[r]
    s = WCOLS + 2 * c0
    return s, s + w


def _ylo(r):
    """start col of range r's [m0|m1] block in yt."""
    return 2 * MM_RANGES[r][0]


def _build_module() -> bass.Bass:
    in_dt = mybir.dt.bfloat16

    nc = bacc.Bacc(None, target_bir_lowering=False, enable_partition_id=False)
    xin = nc.dram_tensor("xin", [P, XIN_COLS], in_dt, kind="ExternalInput")
    yt = nc.dram_tensor("yt", [P, YT_COLS], mybir.dt.bfloat16, kind="ExternalOutput")

    with tile.TileContext(nc) as tc:
        with (
            tc.tile_pool(name="sbpool", bufs=1) as sbpool,
            tc.tile_pool(name="pspool", bufs=1, space="PSUM") as pspool,
        ):
            # Scratch operands for PE warm-up (values irrelevant, but Tile
            # requires a write). DVE memset is fast and DVE is otherwise
            # idle here, so the warm-up matmuls start ~0.5us earlier than
            # with the gpsimd memset, pulling the HW clock boost forward.
            wu = sbpool.tile([P, 512], mybir.dt.bfloat16, name="wu", tag="wu")
            nc.vector.memset(wu[:], 0.0)
            wu_mm = wu[:]

            x_sb = sbpool.tile([P, XIN_COLS], in_dt, name="x", tag="x")
            y_sb = sbpool.tile([P, YT_COLS], mybir.dt.bfloat16, name="y", tag="y")

            # Input chunk DMAs in need-order on the sync HWDGE queue.
            for lo, hi in CHUNKS:
                nc.sync.dma_start(x_sb[:, lo:hi], xin[:, lo:hi])

            ps = [
                [
                    pspool.tile([P, 512], mybir.dt.float32, name=f"ps{m}_{j}", tag=f"ps{m}_{j}")
                    for j in range(4)
                ]
                for m in range(MC)
            ]

            def ps_slice(m, r):
                _, w, j, lo = MM_RANGES[r]
                return ps[m][j][:, lo : lo + w]

            # PE clock warm-up on scratch data while the first chunk is in
            # flight (targets ps[1][3]: real accumulation there starts last
            # among full banks; Tile's WAW tracking keeps program order).
            for wfree in WARMUPS:
                nc.tensor.matmul(
                    ps[1][3][:, :wfree], wu_mm[:, :P], wu_mm[:, :wfree],
                    start=True, stop=True,
                )

            def filler(r, count):
                # Keep the PE busy during input-wait gaps (256-free, cheap).
                # Target a bank that is not mid-accumulation at that point
                # in program order.
                if r <= 2:
                    tgt = ps[1][3][:, 0:256]               # bank3 still unused
                elif r == 3:
                    tgt = ps[0][0][:, 0:256]               # r0 m0 evicted
                else:
                    tgt = ps[0][1][:, 0:256]               # r1 m0 evicted
                for _ in range(count):
                    nc.tensor.matmul(tgt, wu_mm[:, :P], wu_mm[:, :256], start=True, stop=True)

            for r, (c0, w, _, _) in enumerate(MM_RANGES):
                k0c, k1c = _xcols(r)
                filler(r, FILLERS[r])
                for k, kc in enumerate((k0c, k1c)):
                    for m in range(MC):
                        nc.tensor.matmul(
                            ps_slice(m, r),
                            x_sb[:, k * F_OUT + m * P : k * F_OUT + (m + 1) * P],
                            x_sb[:, kc : kc + w],
                            start=(k == 0),
                            stop=(k == KC - 1),
                        )
                # Evict fp32 PSUM -> bf16 SBUF: DVE does m0, ACT does m1 in
                # parallel; one output DMA per range for the packed [m0|m1]
                # block, queued on sync's HWDGE queue (FIFO behind the
                # input chunks = input keeps strict bus priority). The last
                # range's DMA is issued by ACT itself right after its final
                # evict, so it does not queue behind sync's earlier output
                # issues on the tail's critical chain.
                ylo = _ylo(r)
                nc.vector.tensor_copy(y_sb[:, ylo : ylo + w], ps_slice(0, r))
                nc.scalar.copy(y_sb[:, ylo + w : ylo + 2 * w], ps_slice(1, r))
                dst = yt[:, ylo : ylo + 2 * w]
                src = y_sb[:, ylo : ylo + 2 * w]
                if r == len(MM_RANGES) - 1:
                    nc.scalar.dma_start(dst, src)
                else:
                    nc.sync.dma_start(dst, src)
    nc.compile()
    return nc


def _get_module() -> bass.Bass:
    if "m" not in _module_cache:
        _module_cache["m"] = _build_module()
    return _module_cache["m"]


def kernel(h: np.ndarray, adj: np.ndarray, W: np.ndarray, **_unused) -> np.ndarray:
    global LAST_RESULTS
    import ml_dtypes

    bf16 = ml_dtypes.bfloat16
    h = np.asarray(h, dtype=np.float32)
    W = np.asarray(W, dtype=np.float32)
    # Wc[f, head*64+d] = W[head, f, d]
    wc = np.ascontiguousarray(W.transpose(1, 0, 2).reshape(F_IN, F_OUT)).astype(bf16)

    in_maps = []
    for b in range(B):
        xt = h[b].T.astype(bf16)  # [256 f, 2048 n]
        xin = np.empty((P, XIN_COLS), dtype=bf16)
        xin[:, 0:F_OUT] = wc[0:P]
        xin[:, F_OUT : 2 * F_OUT] = wc[P : 2 * P]
        for r, (c0, w, _, _) in enumerate(MM_RANGES):
            s, s1 = _xcols(r)
            xin[:, s : s + w] = xt[0:P, c0 : c0 + w]
            xin[:, s1 : s1 + w] = xt[P : 2 * P, c0 : c0 + w]
        in_maps.append({"xin": xin})

    nc = _get_module()
    res = run_bass_kernel_spmd(nc, in_maps, core_ids=list(range(B)))
    LAST_RESULTS = res

    out = np.empty((B, N, F_OUT), dtype=np.float32)
    yt_full = np.empty((F_OUT, N), dtype=np.float32)
    for b in range(B):
        ytb = res.results[b]["yt"]
        for r, (c0, w, _, _) in enumerate(MM_RANGES):
            ylo = _ylo(r)
            blk = ytb[:, ylo : ylo + 2 * w].astype(np.float32)
            yt_full[0:P, c0 : c0 + w] = blk[:, 0:w]
            yt_full[P : 2 * P, c0 : c0 + w] = blk[:, w : 2 * w]
        out[b] = yt_full.T
    return out


# revision 19
# speedup vs baseline: 1.4823x; 1.1840x over previous
"""Multi-head graph-attention layer for Trainium2 (8-core SPMD).

The reference computes per-head projections hp = einsum("bnf,hfd->bhnd", h, W),
dense attention scores e = hp @ hp^T, LeakyReLU, softmax over the last axis,
and then multiplies hp by sum_j(softmax(e))_j. The sum of a softmax over its
own normalization axis is identically 1, so the layer's exact mathematical
output is hp itself (concatenated over heads):

    out[b, n, h*64+d] = sum_f h[b,n,f] * W[h,f,d]  =  (h[b] @ Wc)[n, h*64+d]

with Wc[f, h*64+d] = W[h,f,d]. `adj` is unused by the reference and ignored.

Sharding: data-parallel over the batch dim B=8, one graph per NeuronCore.
Each core computes Y[b]^T = (Wc^T @ h[b]^T) as a [256,256] x [256,2048]
matmul in bf16 (measured rel err ~2.8e-3 vs the 2e-2 gate), halving both
input and output DMA bytes vs fp32: the kernel is DMA-bound.

HW model distilled from traces (per core):
- DMA engines process ~one descriptor-packet per ~80ns each; packets cap
  at 2048 B, so rows that are multiples of 2048 B move at the full
  ~400 GB/s aggregate while shorter rows waste packet slots. Every DMA
  also costs ~0.6us sequencer issue + ~0.7us DGE start latency + ~0.9us
  completion-semaphore propagation, and >=128 descriptors (~0.64us).
- PE reaches 2.4 GHz only after ~3us of continuous busy (cold 2-4x
  slower), hence scratch warm-up/filler matmuls.

Pipeline (per core):
- Host packs one [128, 4608] bf16 input: Wc's two 128-row k-chunks in
  cols [0:512), then per node-range r its Xk0|Xk1 blocks side by side,
  so one chunk DMA delivers everything a matmul group needs. Chunks are
  1024 cols (2048 B rows) in need-order on the sync HWDGE queue (FIFO =
  input keeps bus priority over the outputs queued behind it).
- Node ranges 256/512/512/512/256: the small head range starts the PE
  ~0.4us earlier; the small tail range shortens the last
  mm->evict->issue->transfer chain. Ranges 0 and 4 share a PSUM bank
  per m so 2m x 4 banks exactly fits PSUM.
- PSUM eviction converts fp32->bf16 on the fly: DVE evicts m0, ACT
  evicts m1 in parallel into a shared [128,4096] bf16 y tile; sync
  issues the first four output DMAs, ACT issues the last one itself
  right after its final evict (skips queueing behind sync).
"""

import numpy as np

import concourse.bass as bass
import concourse.mybir as mybir
import concourse.tile as tile
from concourse import bacc
from concourse.bass_utils import run_bass_kernel_spmd

B = 8          # graphs == cores
N = 2048       # nodes per graph
F_IN = 256     # input features (= contraction dim K)
F_OUT = 256    # num_heads * d_head
P = 128        # SBUF/PSUM partitions
KC = 2         # contraction chunks (256 = 2*128)
MC = 2         # output-feature chunks (256 = 2*128)

WCOLS = 512            # cols [0:512) hold Wc k0|k1
XIN_COLS = WCOLS + 2 * N   # 4608
YT_COLS = 2 * N            # 4096

# (node_c0, width, psum_bank, psum_lo): compute order. Ranges 0 and 4
# share psum bank 0 ([0:256) and [256:512)).
MM_RANGES = [
    (0, 256, 0, 0),
    (256, 512, 1, 0),
    (768, 512, 2, 0),
    (1280, 512, 3, 0),
    (1792, 256, 0, 256),
]
# input DMA chunk column bounds (xin cols); chunk i feeds range i
CHUNKS = [(0, 1024), (1024, 2048), (2048, 3072), (3072, 4096), (4096, 4608)]

# scratch matmul free-dims covering the first input-DMA wait; the HW clock
# manager boosts the PE only after ~4.5us of sustained activity, so start
# as early as possible and stay busy until the first chunk lands.
WARMUPS = [512, 512, 512, 512, 256]
FILLERS = [0, 0, 0, 0, 0]  # fillers hurt PE-bound (cold-clock) runs; disabled

_module_cache = {}

# test.py reads this after calling kernel() to get profile/exec-time info.
LAST_RESULTS = None


def _xcols(r):
    """(k0_col, k1_col) start columns of range r's X blocks in xin."""
    c0, w, _, _ = MM_RANGES[r]
    s = WCOLS + 2 * c0
    return s, s + w


def _ylo(r):
    """start col of range r's [m0|m1] block in yt."""
    return 2 * MM_RANGES[r][0]


def _build_module() -> bass.Bass:
    in_dt = mybir.dt.bfloat16

    nc = bacc.Bacc(None, target_bir_lowering=False, enable_partition_id=False)
    xin = nc.dram_tensor("xin", [P, XIN_COLS], in_dt, kind="ExternalInput")
    yt = nc.dram_tensor("yt", [P, YT_COLS], mybir.dt.bfloat16, kind="ExternalOutput")

    with tile.TileContext(nc) as tc:
        with (
            tc.tile_pool(name="sbpool", bufs=1) as sbpool,
            tc.tile_pool(name="pspool", bufs=1, space="PSUM") as pspool,
        ):
            # Scratch operands for PE warm-up (values irrelevant, but Tile
            # requires a write). DVE memset is fast and DVE is otherwise
            # idle here, so the warm-up matmuls start ~0.5us earlier than
            # with the gpsimd memset, pulling the HW clock boost forward.
            wu = sbpool.tile([P, 512], mybir.dt.bfloat16, name="wu", tag="wu")
            nc.vector.memset(wu[:], 0.0)
            wu_mm = wu[:]

            x_sb = sbpool.tile([P, XIN_COLS], in_dt, name="x", tag="x")
            y_sb = sbpool.tile([P, YT_COLS], mybir.dt.bfloat16, name="y", tag="y")

            # Input chunk DMAs in need-order on the sync HWDGE queue.
            for lo, hi in CHUNKS:
                nc.sync.dma_start(x_sb[:, lo:hi], xin[:, lo:hi])

            ps = [
                [
                    pspool.tile([P, 512], mybir.dt.float32, name=f"ps{m}_{j}", tag=f"ps{m}_{j}")
                    for j in range(4)
                ]
                for m in range(MC)
            ]

            def ps_slice(m, r):
                _, w, j, lo = MM_RANGES[r]
                return ps[m][j][:, lo : lo + w]

            # PE clock warm-up on scratch data while the first chunk is in
            # flight (targets ps[1][3]: real accumulation there starts last
            # among full banks; Tile's WAW tracking keeps program order).
            for wfree in WARMUPS:
                nc.tensor.matmul(
                    ps[1][3][:, :wfree], wu_mm[:, :P], wu_mm[:, :wfree],
                    start=True, stop=True,
                )

            def filler(r, count):
                # Keep the PE busy during input-wait gaps (256-free, cheap).
                # Target a bank that is not mid-accumulation at that point
                # in program order.
                if r <= 2:
                    tgt = ps[1][3][:, 0:256]               # bank3 still unused
                elif r == 3:
                    tgt = ps[0][0][:, 0:256]               # r0 m0 evicted
                else:
                    tgt = ps[0][1][:, 0:256]               # r1 m0 evicted
                for _ in range(count):
                    nc.tensor.matmul(tgt, wu_mm[:, :P], wu_mm[:, :256], start=True, stop=True)

            for r, (c0, w, _, _) in enumerate(MM_RANGES):
                k0c, k1c = _xcols(r)
                filler(r, FILLERS[r])
                for k, kc in enumerate((k0c, k1c)):
                    for m in range(MC):
                        nc.tensor.matmul(
                            ps_slice(m, r),
                            x_sb[:, k * F_OUT + m * P : k * F_OUT + (m + 1) * P],
                            x_sb[:, kc : kc + w],
                            start=(k == 0),
                            stop=(k == KC - 1),
                        )
                # Evict fp32 PSUM -> bf16 SBUF: DVE does m0, ACT does m1 in
                # parallel; one output DMA per range for the packed [m0|m1]
                # block, queued on sync's HWDGE queue (FIFO behind the
                # input chunks = input keeps strict bus priority). The last
                # range's DMA is issued by ACT itself right after its final
                # evict, so it does not queue behind sync's earlier output
                # issues on the tail's critical chain.
                ylo = _ylo(r)
                nc.vector.tensor_copy(y_sb[:, ylo : ylo + w], ps_slice(0, r))
                nc.scalar.copy(y_sb[:, ylo + w : ylo + 2 * w], ps_slice(1, r))
                dst = yt[:, ylo : ylo + 2 * w]
                src = y_sb[:, ylo : ylo + 2 * w]
                if r == len(MM_RANGES) - 1:
                    nc.scalar.dma_start(dst, src)
                else:
                    nc.sync.dma_start(dst, src)
    nc.compile()
    return nc


def _get_module() -> bass.Bass:
    if "m" not in _module_cache:
        _module_cache["m"] = _build_module()
    return _module_cache["m"]


def kernel(h: np.ndarray, adj: np.ndarray, W: np.ndarray, **_unused) -> np.ndarray:
    global LAST_RESULTS
    import ml_dtypes

    bf16 = ml_dtypes.bfloat16
    h = np.asarray(h, dtype=np.float32)
    W = np.asarray(W, dtype=np.float32)
    # Wc[f, head*64+d] = W[head, f, d]
    wc = np.ascontiguousarray(W.transpose(1, 0, 2).reshape(F_IN, F_OUT)).astype(bf16)

    in_maps = []
    for b in range(B):
        xt = h[b].T.astype(bf16)  # [256 f, 2048 n]
        xin = np.empty((P, XIN_COLS), dtype=bf16)
        xin[:, 0:F_OUT] = wc[0:P]
        xin[:, F_OUT : 2 * F_OUT] = wc[P : 2 * P]
        for r, (c0, w, _, _) in enumerate(MM_RANGES):
            s, s1 = _xcols(r)
            xin[:, s : s + w] = xt[0:P, c0 : c0 + w]
            xin[:, s1 : s1 + w] = xt[P : 2 * P, c0 : c0 + w]
        in_maps.append({"xin": xin})

    nc = _get_module()
    res = run_bass_kernel_spmd(nc, in_maps, core_ids=list(range(B)))
    LAST_RESULTS = res

    out = np.empty((B, N, F_OUT), dtype=np.float32)
    yt_full = np.empty((F_OUT, N), dtype=np.float32)
    for b in range(B):
        ytb = res.results[b]["yt"]
        for r, (c0, w, _, _) in enumerate(MM_RANGES):
            ylo = _ylo(r)
            blk = ytb[:, ylo : ylo + 2 * w].astype(np.float32)
            yt_full[0:P, c0 : c0 + w] = blk[:, 0:w]
            yt_full[P : 2 * P, c0 : c0 + w] = blk[:, w : 2 * w]
        out[b] = yt_full.T
    return out
